# revision 14
# baseline (speedup 1.0000x reference)
"""Trainium2 Bass kernel for DeepICD candidate attention.

Reference computation (per batch b):
    S     = X[b] @ a_w                      [L, OS]     (a_b drops out of softmax)
    alpha = softmax(S, axis=L)
    Xp    = alpha^T @ X[b]                  [OS, D]
    Xph   = Xp @ hw_eff + hb_eff            [OS, LAB]   (BN folded into hw/hb on host)
    Xpf   = relu(Xph)
    bLV   = labDescVec[candidate[b]]        [NC, LAB]
    sc    = Xpf @ bLV^T                     [OS, NC]
    a2    = softmax(sc, axis=OS)            (skip-max: |sc| < 13, exp is safe)
    out   = a2^T @ Xpf                      [NC, LAB]

Sharding: data-parallel over batch B=16 across 8 NeuronCores (2 batches/core);
weights and labDescVec replicated; X staged in bf16 (the device matmuls are
bf16 anyway, so this halves HBM traffic with zero numerical change).

Layout strategy: the two per-core batches are MERGED on the partition dim
(batch 0 -> partitions 0:64, batch 1 -> partitions 64:128) everywhere the
natural row count is OS=64.  All M=64 matmuls are issued as column-tiled
pairs (tile_position=(0,0)/(0,64)) so both batches stream through the PE
array concurrently.  S is computed transposed (S^T = a_w^T @ X^T) with the
tiny a_w chunks stationary, so the softmax partition function falls out of
the exp evacuation's accum_out for free; exp(S^T) is transposed back on PE
in merged [128,128] blocks.  X^T itself comes from PE transposes (DMA has
no spare fabric bandwidth).  softmax2 skips max-subtraction and its
normalizer is computed with a ones-matmul that shares the stationary
operand with the final output matmul.
"""

import numpy as np

P = 128
NB = 2          # batches per core
L = 2048
D = 1024
OS = 64
NCC = 256       # candidates per sample
LAB = 1024
CLS = 8921
NT = L // P     # 16 l-tiles
DC = D // P     # 8 d-chunks
HC = LAB // P   # 8 h-chunks
CC = NCC // P   # 2 candidate chunks
NLB = 4         # l-blocks (512 rows each) for the S^T psum
N_CORES = 8
BN_EPS = 1e-5

WARMUP_MM = 40      # dummy matmuls to lift the PE HAM clock-gate before real work
DMAT_BLVT = True    # transpose gathered label rows via the DMA XBAR (else PE)
DMAT_XT = False     # transpose X via the DMA XBAR (else PE): the XBAR op's
                    # deps are not tracked by tile on this stack -> races

_PROG = None


def _build_program():
    import concourse.bass as bass
    import concourse.bacc as bacc
    import concourse.tile as tile
    from concourse import mybir
    from concourse.masks import make_identity

    f32 = mybir.dt.float32
    bf16 = mybir.dt.bfloat16
    i32 = mybir.dt.int32
    AF = mybir.ActivationFunctionType

    nc = bacc.Bacc("TRN2", target_bir_lowering=False, debug=False,
                   num_devices=N_CORES)
    X = nc.dram_tensor("X", [NB, L, D], bf16, kind="ExternalInput")
    cand = nc.dram_tensor("cand", [NB, NCC], i32, kind="ExternalInput")
    aw = nc.dram_tensor("aw", [D, OS], bf16, kind="ExternalInput")
    hw = nc.dram_tensor("hw", [D, LAB], bf16, kind="ExternalInput")
    hb = nc.dram_tensor("hb", [LAB], bf16, kind="ExternalInput")
    lab = nc.dram_tensor("lab", [CLS, LAB], bf16, kind="ExternalInput")
    out_d = nc.dram_tensor("out", [NB, NCC, LAB], bf16, kind="ExternalOutput")

    with tile.TileContext(nc) as tc:
        with (
            tc.tile_pool(name="singles", bufs=1) as singles,
            tc.tile_pool(name="outp", bufs=4) as outp,
            tc.tile_pool(name="pst", bufs=2, space="PSUM") as pst,
            tc.tile_pool(name="pacc", bufs=1, space="PSUM") as pacc,
            tc.tile_pool(name="ptp", bufs=2, space="PSUM") as ptp,
            tc.tile_pool(name="ps2t", bufs=1, space="PSUM") as ps2t,
            tc.tile_pool(name="pdout", bufs=1, space="PSUM") as pdout,
        ):
            # ---- constants ----
            ident = singles.tile([P, P], bf16)
            make_identity(nc, ident[:])
            ones_col = singles.tile([P, 1], bf16)
            nc.vector.memset(ones_col[:], 1.0)
            ones_row = singles.tile([1, P], bf16)
            nc.vector.memset(ones_row[:], 1.0)

            # ---- PE warm-up: the HAM clock gate releases after ~3.4us of
            # sustained activity; burn that window on dummy matmuls while the
            # first X chunks stream in (they have no data dependencies).
            for wi in range(WARMUP_MM):
                wt = pdout.tile([P, 512], f32, tag="dout", name=f"wu_{wi}")
                nc.tensor.matmul(out=wt[:, 0:P], lhsT=ident[:], rhs=ident[:],
                                 start=True, stop=True)

            # ---- small loads (sync queue) ----
            cand_sb = singles.tile([P, NB, CC], i32)
            nc.sync.dma_start(
                out=cand_sb[:], in_=cand[:, :].rearrange("b (c p) -> p b c", p=P)
            )
            aw_sb = singles.tile([P, DC, OS], bf16)
            nc.sync.dma_start(
                out=aw_sb[:], in_=aw[:, :].rearrange("(c p) o -> p c o", p=P)
            )
            hb_sb = singles.tile([1, LAB], bf16)
            nc.sync.dma_start(out=hb_sb[:], in_=hb[None, :])

            # ---- candidate row gathers (gpsimd SWDGE: slow first byte, so
            # issue immediately) ----
            blv = {}
            for b in range(NB):
                for cc in range(CC):
                    g = singles.tile([P, LAB], bf16, name=f"blv_{b}_{cc}")
                    nc.gpsimd.indirect_dma_start(
                        out=g[:], out_offset=None, in_=lab[:, :],
                        in_offset=bass.IndirectOffsetOnAxis(
                            ap=cand_sb[:, b, cc:cc + 1], axis=0,
                        ),
                    )
                    blv[b, cc] = g

            # ---- bulk weight load (scalar queue, overlaps X loads) ----
            hw_sb = singles.tile([P, DC, LAB], bf16)
            nc.scalar.dma_start(
                out=hw_sb[:], in_=hw[:, :].rearrange("(c p) h -> p c h", p=P)
            )

            # ---- X loads: one chunk per (batch, l-block), interleaved so
            # both batches' early tiles arrive first (sync queue).  With
            # DMAT_XT each chunk is immediately re-transposed through the
            # DMA XBAR (one instruction per chunk, alternating HWDGE rings).
            x_sb = [singles.tile([P, NT, D], bf16, name=f"x_{b}")
                    for b in range(NB)]
            xT_sb = [singles.tile([P, NT, DC, P], bf16, name=f"xT_{b}")
                     for b in range(NB)]
            for ch in range(4):
                for b in range(NB):
                    nc.sync.dma_start(
                        out=x_sb[b][:, ch * 4:(ch + 1) * 4, :],
                        in_=X[b, ch * 4 * P:(ch + 1) * 4 * P, :].rearrange(
                            "(t p) d -> p t d", p=P
                        ),
                    )
                    if DMAT_XT:
                        # XBAR-transpose straight from DRAM: an SBUF source
                        # races with its own in-flight load (the transpose's
                        # input deps are not awaited — verified on HW), but a
                        # DRAM input is settled before the kernel starts.
                        # Costs a second HBM read of X; frees PE + DVE.
                        for lt in range(ch * 4, ch * 4 + 4):
                            eng = nc.scalar if (b + lt) % 2 == 0 else nc.sync
                            eng.dma_start_transpose(
                                out=xT_sb[b][:, lt],
                                in_=X[b, lt * P:(lt + 1) * P, :],
                            )

            # ---- phase A, pipelined per l-block (512 rows) ----
            # st holds S^T for the block, both batches merged on partitions.
            et_sb = singles.tile([P, L], bf16)
            e_sb = singles.tile([P, NT, P], bf16)
            z4 = singles.tile([P, NLB], f32)
            xpu = pacc.tile([P, D], f32, tag="acc")

            for lb in range(NLB):
                lts = range(4 * lb, 4 * lb + 4)
                # X^T for this block: PE transposes (8 per (b, l-tile)),
                # evacuated per tile so DVE trails PE by one group
                if not DMAT_XT:
                    for b in range(NB):
                        for lt in lts:
                            tp = ptp.tile([P, DC, P], bf16, tag="tp")
                            for c in range(DC):
                                nc.tensor.transpose(
                                    out=tp[:, c, :],
                                    in_=x_sb[b][:, lt, c * P:(c + 1) * P],
                                    identity=ident[:],
                                )
                            # alternate the evacuation engine so the PE->evac
                            # ping-pong through the psum pool never stalls on
                            # a single engine's queue
                            if (b + lt) % 2 == 0:
                                nc.vector.tensor_copy(out=xT_sb[b][:, lt],
                                                      in_=tp[:])
                            else:
                                nc.scalar.copy(out=xT_sb[b][:, lt], in_=tp[:])
                # S^T block: stationary a_w chunks, col-tiled batch pair
                st = pst.tile([P, 512], f32, tag="st")
                for c in range(DC):
                    for b in range(NB):
                        nc.tensor.matmul(
                            out=st[64 * b:64 * b + 64, :],
                            lhsT=aw_sb[:, c, :],
                            rhs=xT_sb[b][:, 4 * lb:4 * lb + 4, c, :],
                            start=(c == 0), stop=(c == DC - 1),
                            tile_position=(0, 64 * b),
                            skip_group_check=True,
                        )
                # exp + partition-function accumulation
                nc.scalar.activation(
                    out=et_sb[:, 512 * lb:512 * (lb + 1)], in_=st[:],
                    func=AF.Exp, accum_out=z4[:, lb:lb + 1],
                )
                # E^T -> e transposes, both batches per [128,128] block
                tp = ptp.tile([P, DC, P], bf16, tag="tp")
                for j, lt in enumerate(lts):
                    nc.tensor.transpose(
                        out=tp[:, j, :],
                        in_=et_sb[:, lt * P:(lt + 1) * P],
                        identity=ident[:],
                    )
                if lb % 2 == 0:
                    nc.vector.tensor_copy(
                        out=e_sb[:, 4 * lb:4 * lb + 4, :], in_=tp[:, 0:4, :]
                    )
                else:
                    nc.scalar.copy(
                        out=e_sb[:, 4 * lb:4 * lb + 4, :], in_=tp[:, 0:4, :]
                    )
                # Xpu accumulation (unnormalized), col-tiled batch pair
                for lt in lts:
                    for nh in range(2):
                        for b in range(NB):
                            nc.tensor.matmul(
                                out=xpu[64 * b:64 * b + 64,
                                        nh * 512:(nh + 1) * 512],
                                lhsT=e_sb[:, lt, 64 * b:64 * b + 64],
                                rhs=x_sb[b][:, lt, nh * 512:(nh + 1) * 512],
                                start=(lt == 0), stop=(lt == NT - 1),
                                tile_position=(0, 64 * b),
                                skip_group_check=True,
                            )

            # softmax normalizer: z = sum of the 4 block partials
            z = singles.tile([P, 1], f32)
            nc.vector.tensor_reduce(
                out=z[:], in_=z4[:], axis=mybir.AxisListType.X,
                op=mybir.AluOpType.add,
            )
            rz = singles.tile([P, 1], f32)
            nc.vector.reciprocal(out=rz[:], in_=z[:])

            # evacuate Xp = Xpu/z (normalize here: partitions are (b, os),
            # exactly matching rz)
            xpu_sb = singles.tile([P, D], bf16)
            nc.scalar.activation(out=xpu_sb[:], in_=xpu[:], func=AF.Copy,
                                 scale=rz[:])
            # Xp^T (d on partitions, merged (b, os) on free)
            xput_sb = singles.tile([P, DC, P], bf16)
            tp = ptp.tile([P, DC, P], bf16, tag="tp")
            for c in range(DC):
                nc.tensor.transpose(
                    out=tp[:, c, :], in_=xpu_sb[:, c * P:(c + 1) * P],
                    identity=ident[:],
                )
            nc.vector.tensor_copy(out=xput_sb[:], in_=tp[:])

            # ---- phase B: Xpf = relu(Xp @ hw + hb), col-tiled batch pair ----
            xph = pacc.tile([P, LAB], f32, tag="acc")
            for c in range(DC):
                for nh in range(2):
                    for b in range(NB):
                        nc.tensor.matmul(
                            out=xph[64 * b:64 * b + 64, nh * 512:(nh + 1) * 512],
                            lhsT=xput_sb[:, c, 64 * b:64 * b + 64],
                            rhs=hw_sb[:, c, nh * 512:(nh + 1) * 512],
                            start=(c == 0), stop=False,
                            tile_position=(0, 64 * b),
                            skip_group_check=True,
                        )
            for nh in range(2):
                for b in range(NB):
                    nc.tensor.matmul(
                        out=xph[64 * b:64 * b + 64, nh * 512:(nh + 1) * 512],
                        lhsT=ones_row[:, 64 * b:64 * b + 64],
                        rhs=hb_sb[:, nh * 512:(nh + 1) * 512],
                        start=False, stop=True,
                        tile_position=(0, 64 * b),
                        skip_group_check=True,
                    )
            xpf_sb = singles.tile([P, LAB], bf16)
            nc.scalar.activation(out=xpf_sb[:], in_=xph[:], func=AF.Relu)

            # Xpf^T (h on partitions, merged (b, os) on free)
            xpft_sb = singles.tile([P, HC, P], bf16)
            tp = ptp.tile([P, HC, P], bf16, tag="tp")
            for hc in range(HC):
                nc.tensor.transpose(
                    out=tp[:, hc, :], in_=xpf_sb[:, hc * P:(hc + 1) * P],
                    identity=ident[:],
                )
            nc.vector.tensor_copy(out=xpft_sb[:], in_=tp[:])

            # ---- candidate label rows transposed (h on partitions) ----
            blvT = []
            for b in range(NB):
                t = singles.tile([P, CC, HC, P], bf16, name=f"blvT_{b}")
                for cc in range(CC):
                    if DMAT_BLVT:
                        eng = nc.scalar if cc == 0 else nc.sync
                        eng.dma_start_transpose(
                            out=t[:, cc], in_=blv[b, cc][:],
                        )
                    else:
                        tpb = ptp.tile([P, HC, P], bf16, tag="tp")
                        for hc in range(HC):
                            nc.tensor.transpose(
                                out=tpb[:, hc, :],
                                in_=blv[b, cc][:, hc * P:(hc + 1) * P],
                                identity=ident[:],
                            )
                        nc.vector.tensor_copy(out=t[:, cc], in_=tpb[:])
                blvT.append(t)

            # ---- phase C: scores transposed, col-tiled batch pair ----
            # s2t[64b+o, 128cc+c] = sum_h Xpf[b,o,h] * blv[b,cc*128+c,h]
            s2t = ps2t.tile([P, NCC + 2 * NB * CC], f32, tag="s2t")
            for hc in range(HC):
                for b in range(NB):
                    nc.tensor.matmul(
                        out=s2t[64 * b:64 * b + 64, 0:NCC],
                        lhsT=xpft_sb[:, hc, 64 * b:64 * b + 64],
                        rhs=blvT[b][:, :, hc, :],
                        start=(hc == 0), stop=(hc == HC - 1),
                        tile_position=(0, 64 * b),
                        skip_group_check=True,
                    )
            # skip-max exp: |s2| < 13 (verified against the reference stats)
            e2t_sb = singles.tile([P, NCC], bf16)
            nc.scalar.activation(out=e2t_sb[:], in_=s2t[:, 0:NCC], func=AF.Exp)

            # ---- phase D: out = softmax(s2)^T Xpf; the ones-matmul for the
            # softmax2 normalizer shares its stationary operand with the
            # output matmuls ----
            for b in range(NB):
                for cc in range(CC):
                    j = NCC + 2 * (2 * b + cc)
                    lhs = e2t_sb[64 * b:64 * b + 64, cc * P:(cc + 1) * P]
                    nc.tensor.matmul(
                        out=s2t[:, j:j + 1], lhsT=lhs,
                        rhs=ones_col[64 * b:64 * b + 64, :],
                        start=True, stop=True, skip_group_check=True,
                    )
                    z2c = singles.tile([P, 1], f32, name=f"z2c_{b}_{cc}")
                    nc.vector.tensor_copy(out=z2c[:], in_=s2t[:, j:j + 1])
                    rz2 = singles.tile([P, 1], f32, name=f"rz2_{b}_{cc}")
                    nc.vector.reciprocal(out=rz2[:], in_=z2c[:])
                    for nh in range(2):
                        op = pdout.tile([P, 512], f32, tag="dout")
                        nc.tensor.matmul(
                            out=op[:], lhsT=lhs,
                            rhs=xpf_sb[64 * b:64 * b + 64,
                                       nh * 512:(nh + 1) * 512],
                            start=True, stop=True, skip_group_check=True,
                        )
                        ob = outp.tile([P, 512], bf16, tag="ob")
                        if nh == 0:
                            nc.vector.tensor_scalar(
                                out=ob[:], in0=op[:], scalar1=rz2[:],
                                scalar2=None, op0=mybir.AluOpType.mult,
                            )
                        else:
                            nc.scalar.activation(
                                out=ob[:], in_=op[:], func=AF.Copy,
                                scale=rz2[:],
                            )
                        nc.sync.dma_start(
                            out=out_d[b, cc * P:(cc + 1) * P,
                                      nh * 512:(nh + 1) * 512],
                            in_=ob[:],
                        )
    nc.finalize()
    return nc


def _ensure_neuron_platform():
    # The kernel must execute on the axon-tunneled NeuronCores; a stray
    # JAX_PLATFORMS=cpu pin (common for running the jax reference) would
    # hide them from PJRT. Only act if jax hasn't initialized a backend yet.
    import os
    import sys

    if os.environ.get("JAX_PLATFORMS") == "cpu":
        jax = sys.modules.get("jax")
        initialized = False
        if jax is not None:
            try:
                from jax._src import xla_bridge

                initialized = xla_bridge.backends_are_initialized()
            except Exception:
                initialized = False
        if not initialized:
            del os.environ["JAX_PLATFORMS"]


def _get_program():
    global _PROG
    if _PROG is None:
        _ensure_neuron_platform()
        _PROG = _build_program()
    return _PROG


def _make_in_maps(inputs):
    import ml_dtypes

    bf16 = ml_dtypes.bfloat16
    X = np.ascontiguousarray(
        np.asarray(inputs["X"], dtype=np.float32).astype(bf16)
    )
    cand = np.ascontiguousarray(
        np.asarray(inputs["candidate"]).astype(np.int32)
    )
    a_w = np.asarray(inputs["a_w"], dtype=np.float32)
    h_w = np.asarray(inputs["h_w"], dtype=np.float32)
    h_b = np.asarray(inputs["h_b"], dtype=np.float32)
    g = np.asarray(inputs["bn_gamma"], dtype=np.float32)
    be = np.asarray(inputs["bn_beta"], dtype=np.float32)
    mu = np.asarray(inputs["bn_mean"], dtype=np.float32)
    var = np.asarray(inputs["bn_var"], dtype=np.float32)
    lab = np.ascontiguousarray(
        np.asarray(inputs["labDescVec"], dtype=np.float32).astype(bf16)
    )

    s = g / np.sqrt(var + BN_EPS)
    hw_eff = np.ascontiguousarray((h_w * s[None, :]).astype(bf16))
    hb_eff = ((h_b - mu) * s + be).astype(bf16)
    aw_bf = np.ascontiguousarray(a_w.astype(bf16))

    in_maps = []
    for ci in range(N_CORES):
        in_maps.append({
            "X": X[ci * NB:(ci + 1) * NB],
            "cand": cand[ci * NB:(ci + 1) * NB],
            "aw": aw_bf,
            "hw": hw_eff,
            "hb": hb_eff,
            "lab": lab,
        })
    return in_maps


def run(inputs, trace=False, tmpdir=None):
    from concourse.bass_utils import run_bass_kernel_spmd

    nc = _get_program()
    in_maps = _make_in_maps(inputs)
    kwargs = {}
    if trace and tmpdir is None:
        tmpdir = "/root/problem/trace_out"
        import os
        import shutil

        shutil.rmtree(tmpdir, ignore_errors=True)
        os.makedirs(tmpdir, exist_ok=True)
    if tmpdir is not None:
        kwargs["tmpdir"] = tmpdir
    res = run_bass_kernel_spmd(
        nc, in_maps, list(range(N_CORES)), trace=trace, **kwargs,
    )
    out = np.concatenate(
        [np.asarray(r["out"], dtype=np.float32) for r in res.results], axis=0
    )
    return out, res


def kernel(**inputs):
    out, _ = run(inputs, trace=False)
    return out
